# revision 38
# baseline (speedup 1.0000x reference)
"""CrossNonLocalBlockND (B=1, Cx=64, Ci=32, D=8, H=W=48) on one TRN2 core.

Strategy: single NeuronCore in fp16 (multi-core SPMD dispatch through the
axon tunnel costs ~0.6ms per extra core per call, so one core wins at this
problem size). The kernel is Activation-engine-bound: 85M softmax exps at
~1 elem/cycle/partition @1.2GHz = ~550us. Everything else (matmuls on PE,
pooling/normalization/BN on DVE) is software-pipelined underneath the
continuous Act exp stream:

 - scores for group i+2 are prefetched while exp(i) runs (double-buffered
   score PSUM), so Act never waits on PE;
 - PV accumulation (with a 1/128-scaled ones-column producing the softmax
   denominator Z/128, so 128/Z stays fp16-normal even for Z ~ 1e6)
   follows exp(i) immediately on PE;
 - the per-query-tile normalization + W projection are spread over kg slots
   (0,2,4,6,8,9,10) of the next query tile so their PE/DVE ops never block
   the exp stream (the fp16 broadcast matmul costs 1 cycle/row, not the
   4x of fp32);
 - BatchNorm statistics (sum from the fp32 W-projection PSUM, sum-of-
   squares from fp16) are accumulated per query tile on DVE during
   attention instead of in a serial tail pass.

Measurement model (validated by microbenches + three kernel variants):
measured pipelined ns/call ~= 935us fixed axon-tunnel dispatch + device
time, and CoreSim's no_exec cost model predicts device time within ~3%.
The device time is Act-bound: 648 exps x (1024+222 access cycles)/1.2GHz
= 673us, plus ~45us phase-A head and ~15us BN tail.

Math folds (exact, done host-side):
 - phi bias: maxpool(phi_w c + phi_b) = maxpool(phi_w c) + phi_b, and a
   per-query-constant score offset is softmax-invariant -> phi_b dropped.
 - g bias: attention rows sum to 1, so g_b shifts y by a per-channel
   constant; W maps it to a per-channel constant on W_y, which training-mode
   BatchNorm subtracts exactly -> g_b dropped.
 - W bias: per-channel constant, removed by BatchNorm -> W_b dropped.
 - theta bias changes scores per-key -> kept (fused into the Act copy that
   moves theta from PSUM to SBUF).

Device layout: input "cxw" fp16 [128, 18432] holds x (rows 0-63) and
context (rows 64-127). Theta is stored 2x-replicated in 64 rows so the
score matmul row-tiles 2 ways (2 k-chunks of 128 keys per 512-query tile,
concurrent on disjoint 32-row PE strips).
"""

import numpy as np

import concourse.bass as bass
import concourse.mybir as mybir
import concourse.tile as tile
from concourse import bacc
from concourse.bass_utils import run_bass_kernel_spmd
from concourse.masks import make_identity

CX, CI, D, H, W = 64, 32, 8, 48, 48
NQ = D * H * W                 # 18432 query positions
NK = D * (H // 2) * (W // 2)   # 4608 key positions (after 2x2 maxpool)
EPS = 1e-5
FP = mybir.dt.float32
HF = mybir.dt.float16
HF_NP = np.float16

QW = 512                 # query tile width
NQT = NQ // QW           # 36 query tiles
KT = 128                 # k-chunk (score-matmul M)
R = 2                    # k-chunks per score group (2-way PE row tiling)
NKG = NK // (R * KT)     # 18 groups of 2 row-tiled k-chunks
TOT = NQT * NKG          # 648 (qi, kg) groups
N_CORES = 1


def build():
    nc = bacc.Bacc("TRN2", target_bir_lowering=False, num_devices=1)

    cxw = nc.dram_tensor("cxw", [128, NQ], HF, kind="ExternalInput")
    wb = nc.dram_tensor("wb", [128, 256], HF, kind="ExternalInput")
    wf = nc.dram_tensor("wf", [128, 4], FP, kind="ExternalInput")
    out = nc.dram_tensor("out", [CX, NQ], HF, kind="ExternalOutput")

    with tile.TileContext(nc) as tc:
        with tc.tile_pool(name="big", bufs=1) as big, \
             tc.tile_pool(name="sm", bufs=1) as sm:
            CXS = big.tile([128, NQ], HF, tag="cxs")     # x rows 0-63, ctx rows 64-127
            TH = big.tile([64, NQ], HF, tag="th")        # theta+theta_b, replicated 2x
            WY = big.tile([CX, NQ], HF, tag="wy")        # W @ y_norm
            wscr = big.tile([CX, NQ // 2], HF, tag="wscr")
            pooled = big.tile([CX, NK], FP, tag="pooled")  # rows 0-31 g, 32-63 phi

            WBs = sm.tile([128, 256], HF, tag="wbs")
            WFs = sm.tile([128, 4], FP, tag="wfs")
            nc.sync.dma_start(out=WBs[:, :], in_=wb[:, :])
            nc.sync.dma_start(out=WFs[:, :], in_=wf[:, :])
            NCH = 4
            for c in range(NCH):
                cc = slice(c * (NQ // NCH), (c + 1) * (NQ // NCH))
                nc.sync.dma_start(out=CXS[:, cc], in_=cxw[:, cc])

            ident = sm.tile([32, 32], FP, tag="ident")
            make_identity(nc, ident[:, :])
            ones33 = sm.tile([1, CI + 1], HF, tag="ones33")
            nc.vector.memset(ones33[:, :], 1.0)

            pgT = WBs[64:128, 0:64]        # cols 0-31 g_w^T, 32-63 phi_w^T
            thT2 = WBs[0:64, 64:128]       # theta_w^T replicated 2x in columns
            wT = WBs[0:32, 192:256]        # W_w^T
            thb = WFs[0:64, 0:1]           # theta_b replicated 2x (fp32)
            gam = WFs[0:CX, 1:2]
            bet = WFs[0:CX, 2:3]

            # ---- theta (Act moves PSUM->SBUF + bias) + phi/g conv + pool ----
            # Processed in two halves (d-slices 0-3 / 4-7) so the W-pool,
            # phi repack, and g-transpose of half 0 overlap half 1's convs.
            phi_rt = sm.tile([R * 32, NKG * KT], HF, tag="phi_rt")
            gxT = sm.tile([128, NK // KT, CI + 1], HF, tag="gxt")
            pv_phi = pooled[CI:2 * CI, :].rearrange(
                "p (kg r c) -> p kg r c", kg=NKG, r=R)
            wv = wscr[:, :].rearrange("p (dh two w) -> p dh two w", two=2, w=24)
            nc.vector.memset(gxT[:, :, CI:CI + 1], 1.0 / 128.0)
            with tc.tile_pool(name="ps_a", bufs=2, space="PSUM") as ps_a, \
                 tc.tile_pool(name="ps_b", bufs=2, space="PSUM") as ps_b:
                def pool_phi(dh0, ndh, kg0, nkg_c):
                    kc = slice(dh0 * 24, (dh0 + ndh) * 24)
                    nc.vector.tensor_max(
                        out=pooled[:, kc],
                        in0=wv[:, dh0:dh0 + ndh, 0, :],
                        in1=wv[:, dh0:dh0 + ndh, 1, :])
                    # phi repack row-tiled: block r holds k-chunk R*kg+r of
                    # group kg in rows 32r..32r+31, cols kg*KT..
                    for r in range(R):
                        nc.vector.tensor_copy(
                            out=phi_rt[32 * r:32 * r + 32,
                                       kg0 * KT:(kg0 + nkg_c) * KT].rearrange(
                                "p (kg k) -> p kg k", kg=nkg_c),
                            in_=pv_phi[:, kg0:kg0 + nkg_c, r, :],
                        )

                def conv(t, pool):
                    cps = pool.tile([CX, QW], FP, tag="cv")
                    nc.tensor.matmul(out=cps[:, :], lhsT=pgT[:, :],
                                     rhs=CXS[64:128, t * QW:(t + 1) * QW],
                                     start=True, stop=True)
                    nc.vector.reduce_max(
                        out=wscr[:, t * (QW // 2):(t + 1) * (QW // 2)],
                        in_=cps[:, :].rearrange("p (a two) -> p a two", two=2),
                        axis=mybir.AxisListType.X,
                    )

                for t in range(36):
                    qc = slice(t * QW, (t + 1) * QW)
                    thps = ps_a.tile([64, QW], FP, tag="th")
                    nc.tensor.matmul(out=thps[:, :], lhsT=thT2[:, :],
                                     rhs=CXS[0:64, qc], start=True, stop=True)
                    nc.scalar.activation(
                        out=TH[:, qc], in_=thps[:, :],
                        func=mybir.ActivationFunctionType.Identity, bias=thb)
                    if t < 18:
                        conv(t, ps_b)
                pool_phi(0, 96, 0, 9)
                # g transposed per k-chunk: [128, 36, 33], col 32 = ones.
                # Half 1 (g 9-17) is emitted inside the attention loop so
                # score(0) isn't queued behind PE ops that wait on half-1
                # pooling -- the exp stream starts ~13us earlier.
                for g in range(9):
                    tps = ps_a.tile([128, 64], FP, tag="tp")
                    for j in range(2):
                        kt = 2 * g + j
                        nc.tensor.transpose(
                            out=tps[:, 32 * j:32 * j + 32],
                            in_=pooled[0:CI, kt * KT:(kt + 1) * KT],
                            identity=ident[:, :],
                        )
                    nc.vector.tensor_copy(
                        out=gxT[:, 2 * g:2 * g + 2, 0:CI],
                        in_=tps[:, :].rearrange("p (j c) -> p j c", j=2),
                    )

            # ---- attention + W projection: flattened software pipeline ----
            s1p = sm.tile([CX, NQT], FP, tag="s1p")
            s2p = sm.tile([CX, NQT], FP, tag="s2p")
            with tc.tile_pool(name="ps_s", bufs=2, space="PSUM") as ps_s, \
                 tc.tile_pool(name="ps_pv", bufs=2, space="PSUM") as ps_pv, \
                 tc.tile_pool(name="ps_m", bufs=1, space="PSUM") as ps_m, \
                 tc.tile_pool(name="ps_g", bufs=1, space="PSUM") as ps_g, \
                 tc.tile_pool(name="ep", bufs=2) as ep, \
                 tc.tile_pool(name="yp", bufs=2) as yp, \
                 tc.tile_pool(name="sqp", bufs=2) as sqp:

                sps_t = [None, None]       # score psum, rotated
                pv_t = [None, None]        # PV accumulator psum per qi parity
                rz_t = {}
                rzp_t = {}
                yn_t = {}
                wps_t = {}

                def score(i):
                    qi, kg = divmod(i, NKG)
                    qc = slice(qi * QW, (qi + 1) * QW)
                    sps = ps_s.tile([128, R * QW], FP, tag="s")
                    sps_t[i % 2] = sps
                    for r in range(R):
                        nc.tensor.matmul(
                            out=sps[:, r * QW:(r + 1) * QW],
                            lhsT=phi_rt[32 * r:32 * r + 32,
                                        kg * KT:(kg + 1) * KT],
                            rhs=TH[32 * r:32 * r + 32, qc],
                            start=True, stop=True,
                            tile_position=(32 * r, 0),
                        )

                NORM_SLOTS = (0, 2, 4, 6, 8, 9, 10)   # kg slots for the steps

                def norm_step(j, step):
                    # normalization chain for query tile j, slotted into the
                    # pipeline so its PE ops never starve the Act stream
                    qc = slice(j * QW, (j + 1) * QW)
                    pv = pv_t[j % 2]
                    if step == 0:
                        rz = yp.tile([1, QW], HF, tag="rz")
                        rz_t[j] = rz
                        with nc.allow_low_precision(
                                reason="1/Z broadcast in fp16; Z ~1e4 max"):
                            nc.vector.reciprocal(out=rz[:, :],
                                                 in_=pv[CI:CI + 1, :])
                    elif step == 1:
                        rzp = ps_m.tile([CI + 1, QW], FP, tag="m")
                        rzp_t[j] = rzp
                        nc.tensor.matmul(out=rzp[:, :], lhsT=ones33[:, :],
                                         rhs=rz_t[j][:, :], start=True, stop=True)
                    elif step == 2:
                        rzs = yp.tile([CI + 1, QW], FP, tag="rzs")
                        rz_t[j] = rzs
                        nc.vector.tensor_copy(out=rzs[:, :], in_=rzp_t[j][:, :])
                    elif step == 3:
                        yn = yp.tile([CI + 1, QW], HF, tag="yn")
                        yn_t[j] = yn
                        nc.vector.scalar_tensor_tensor(
                            out=yn[:, :], in0=pv[:, :], scalar=1.0 / 128.0,
                            in1=rz_t[j][:, :], op0=mybir.AluOpType.mult,
                            op1=mybir.AluOpType.mult)
                    elif step == 4:
                        wps = ps_m.tile([CX, QW], FP, tag="m")
                        wps_t[j] = wps
                        nc.tensor.matmul(out=wps[:, :], lhsT=wT[:, :],
                                         rhs=yn_t[j][0:CI, :], start=True,
                                         stop=True)
                    elif step == 5:
                        wps = wps_t.pop(j)
                        nc.vector.tensor_copy(out=WY[:, qc], in_=wps[:, :])
                        nc.vector.reduce_sum(out=s1p[:, j:j + 1], in_=wps[:, :],
                                             axis=mybir.AxisListType.X)
                        rz_t.pop(j); rzp_t.pop(j); yn_t.pop(j)
                    elif step == 6:
                        sq = sqp.tile([CX, QW], HF, tag="sq")
                        nc.vector.tensor_mul(sq[:, :], WY[:, qc], WY[:, qc])
                        nc.vector.reduce_sum(out=s2p[:, j:j + 1], in_=sq[:, :],
                                             axis=mybir.AxisListType.X)

                score(0)
                score(1)
                for i in range(TOT):
                    qi, kg = divmod(i, NKG)
                    sps = sps_t[i % 2]
                    et = ep.tile([128, R * QW], HF, tag="e")
                    nc.scalar.activation(
                        out=et[:, :], in_=sps[:, :],
                        func=mybir.ActivationFunctionType.Exp,
                    )
                    if i + 2 < TOT:
                        score(i + 2)
                    if kg == 0:
                        pv = ps_pv.tile([CI + 1, QW], FP, tag="pv")
                        pv_t[qi % 2] = pv
                    pv = pv_t[qi % 2]
                    for r in range(R):
                        kt = R * kg + r
                        nc.tensor.matmul(
                            out=pv[:, :],
                            lhsT=gxT[:, kt, :],
                            rhs=et[:, r * QW:(r + 1) * QW],
                            start=(kg == 0 and r == 0),
                            stop=(kg == NKG - 1 and r == R - 1),
                        )
                    if qi == 0:
                        if kg <= 8:
                            conv(18 + 2 * kg, ps_g)
                            conv(19 + 2 * kg, ps_g)
                        if kg in (3, 6, 9):
                            s = kg // 3 - 1
                            pool_phi(96 + 32 * s, 32, 9 + 3 * s, 3)
                    if qi == 0 and 5 <= kg <= 13:
                        g = 9 + (kg - 5)
                        tps = ps_g.tile([128, 64], FP, tag="cv")
                        for j in range(2):
                            kt = 2 * g + j
                            nc.tensor.transpose(
                                out=tps[:, 32 * j:32 * j + 32],
                                in_=pooled[0:CI, kt * KT:(kt + 1) * KT],
                                identity=ident[:, :],
                            )
                        nc.vector.tensor_copy(
                            out=gxT[:, 2 * g:2 * g + 2, 0:CI],
                            in_=tps[:, :].rearrange("p (j c) -> p j c", j=2),
                        )
                    if qi >= 1 and kg in NORM_SLOTS:
                        norm_step(qi - 1, NORM_SLOTS.index(kg))
                for step in range(7):
                    norm_step(NQT - 1, step)

            # ---- BatchNorm finalize (stats already accumulated) + residual --
            s1 = sm.tile([CX, 1], FP, tag="s1")
            s2 = sm.tile([CX, 1], FP, tag="s2")
            nc.vector.reduce_sum(out=s1[:, :], in_=s1p[:, :],
                                 axis=mybir.AxisListType.X)
            nc.vector.reduce_sum(out=s2[:, :], in_=s2p[:, :],
                                 axis=mybir.AxisListType.X)
            with tc.tile_pool(name="bnp", bufs=2) as bnp:
                mu = sm.tile([CX, 1], FP, tag="mu")
                ex2 = sm.tile([CX, 1], FP, tag="ex2")
                var = sm.tile([CX, 1], FP, tag="var")
                a0 = sm.tile([CX, 1], FP, tag="a0")
                av = sm.tile([CX, 1], FP, tag="av")
                bv = sm.tile([CX, 1], FP, tag="bv")
                t0 = sm.tile([CX, 1], FP, tag="t0")
                nc.vector.tensor_scalar_mul(mu[:, :], s1[:, :], 1.0 / NQ)
                nc.vector.tensor_scalar_mul(ex2[:, :], s2[:, :], 1.0 / NQ)
                nc.vector.tensor_mul(t0[:, :], mu[:, :], mu[:, :])
                nc.vector.tensor_sub(var[:, :], ex2[:, :], t0[:, :])
                nc.vector.tensor_scalar_add(var[:, :], var[:, :], EPS)
                # rsqrt(v) = exp(-0.5 * ln(v))
                nc.scalar.activation(out=a0[:, :], in_=var[:, :],
                                     func=mybir.ActivationFunctionType.Ln)
                nc.scalar.activation(out=a0[:, :], in_=a0[:, :],
                                     func=mybir.ActivationFunctionType.Exp,
                                     scale=-0.5)
                nc.vector.tensor_mul(av[:, :], a0[:, :], gam[:, :])
                nc.vector.tensor_mul(t0[:, :], mu[:, :], av[:, :])
                nc.vector.tensor_sub(bv[:, :], bet[:, :], t0[:, :])

                # out = (WY * av + bv) + x, written into the dead ctx rows
                # (both tensor_tensor SBUF inputs must share base partition 0).
                # Split DVE/GPSIMD so the two tail chains run concurrently.
                CH = NQ // 9
                for t in range(9):
                    ch = slice(t * CH, (t + 1) * CH)
                    bn = bnp.tile([CX, CH], HF, tag="bn")
                    nc.vector.tensor_scalar(
                        out=bn[:, :], in0=WY[:, ch], scalar1=av[:, :],
                        scalar2=bv[:, :], op0=mybir.AluOpType.mult,
                        op1=mybir.AluOpType.add,
                    )
                    nc.vector.tensor_add(CXS[64:128, ch], bn[:, :],
                                         CXS[0:CX, ch])
                    nc.sync.dma_start(out=out[:, ch], in_=CXS[64:128, ch])

    nc.finalize()
    return nc


_NC = None


def _get_nc():
    global _NC
    if _NC is None:
        _NC = build()
    return _NC


def build_in_map(x, context, theta_w, theta_b, phi_w, phi_b, g_w, g_b,
                 W_w, W_b, bn_gamma, bn_beta):
    xf = np.asarray(x, np.float32).reshape(CX, NQ)
    cf = np.asarray(context, np.float32).reshape(CX, NQ)
    cxw = np.concatenate([xf, cf], axis=0).astype(HF_NP)

    wbm = np.zeros((128, 256), np.float32)
    wbm[64:128, 0:32] = np.asarray(g_w, np.float32).T
    wbm[64:128, 32:64] = np.asarray(phi_w, np.float32).T
    wbm[0:64, 64:128] = np.tile(np.asarray(theta_w, np.float32).T, (1, 2))
    wbm[0:32, 192:256] = np.asarray(W_w, np.float32).T

    wfm = np.zeros((128, 4), np.float32)
    wfm[0:64, 0] = np.tile(np.asarray(theta_b, np.float32), 2)
    wfm[0:CX, 1] = np.asarray(bn_gamma, np.float32)
    wfm[0:CX, 2] = np.asarray(bn_beta, np.float32)

    return {"cxw": np.ascontiguousarray(cxw),
            "wb": np.ascontiguousarray(wbm.astype(HF_NP)),
            "wf": np.ascontiguousarray(wfm)}


def kernel(x, context, theta_w, theta_b, phi_w, phi_b, g_w, g_b, W_w, W_b,
           bn_gamma, bn_beta):
    nc = _get_nc()
    in_map = build_in_map(x, context, theta_w, theta_b, phi_w, phi_b,
                          g_w, g_b, W_w, W_b, bn_gamma, bn_beta)
    res = run_bass_kernel_spmd(nc, [in_map], core_ids=[0])
    full = np.asarray(res.results[0]["out"], dtype=np.float32)
    return full.reshape(1, CX, D, H, W)
